# revision 1
# baseline (speedup 1.0000x reference)
"""TRN2 Bass kernel v3 for nn_BasicEuclideanDistModel (temporal point-process loss).

Strategy (data-parallel over 8 NeuronCores):
  Host prep (index work + per-TABLE transforms only — no per-event arithmetic):
    - Sort the 8M events by event_time (pure index permutation; the sum is
      permutation invariant), shard contiguous 1M-event slices per core.
      After sorting, the 977 events in one SBUF partition row span a t-range
      of ~1.2e-4, so t is replaced by one per-row value t_p (the row-median
      element) — the t stream disappears from DMA.
    - Per-node tables: 64*z0, 64*v0 (u-side) and negated -64*z0, -64*v0
      (v-side), cast to fp8 e4m3. 8 fp8 streams = 8 B/event.
    - DoubleRow fp8 weight pairs: I2 = [I | I] and D2[tile] = [diag(t_p) |
      diag(t_p)] so ONE DoubleRow matmul computes W.T@a + W.T@b at 0.5
      cycles/row.
  Device per event half-tile [128, ~489] (PSUM-bank sized):
    - PE: ax_psum = I2x(zu|zvn) + D2x(vu|vvn)  (2 DoubleRow matmuls/comp)
    - DVE custom op SQSUM: q = ax^2 + ay^2 straight from PSUM (one op)
    - ACT: Sqrt(q) with fused per-row accumulate -> acc column per tile.
  Pairs (62.5K/core, no t): fp8; DVE adds dz/dv, SQSUM builds A=|dz|^2 and
  C=|dv|^2, 3 ops build B2=2dz.dv; 10 Riemann q_r via independent STT Horner
  steps split across DVE and Pool; one big ACT Sqrt and one big ACT
  Exp(beta - d/64) with fused accumulate, placed mid-stream between event
  tiles so only 3 ACT table loads occur per pass. Pad-pair contribution is a
  known constant, subtracted exactly in the combine.
  Final: host combines [128,12] f32 partials in f64 and unscales by 1/64.
"""
import sys
import numpy as np

sys.path.insert(0, "/opt/trn_rl_repo")

import ml_dtypes  # noqa: E402

BF16 = ml_dtypes.bfloat16
FP8 = ml_dtypes.float8_e4m3

N_POINTS = 100000
N_EVENTS = 8000000
N_PAIRS = 500000
R = 10
N_CORES = 8
SCALE = 64.0

E_CORE = N_EVENTS // N_CORES          # 1,000,000
EV_TILES = 8
EV_N = 977                            # free elems per tile row
E_PAD = EV_TILES * 128 * EV_N         # 1,000,448
HALF0 = 489                           # psum chunk size (fits one 2KB bank)
NEV = 8                               # event streams (no t!)
P_CORE = N_PAIRS // N_CORES           # 62,500
PR_N = (P_CORE + 127) // 128          # 489
NPR = 8
PAD_Z = 240.0                         # pad pairs: dz = 480 -> exp tiny const

# stream order: DoubleRow pairs adjacent (u-side raw, v-side negated)
EV_S = {n: i for i, n in enumerate(
    ["zux", "zvnx", "zuy", "zvny", "vux", "vvnx", "vuy", "vvny"])}

_NC_CACHE = {}
_OPS = None

# ablation flags (sim experiments only)
SKIP_PAIRS = False
SKIP_EVENTS = False
USE_DRSW = False  # both DoubleRow variants crash the NRT exec unit


def _register_ops():
    """Register fused custom DVE ops (runtime append to dve_ops.OPS, shas
    computed from lower() itself):
      SQSUM_ANT:  out = Src0^2 + Src1^2   (both SBUF; pairs A/C)
      SQPLUS_ANT: out = Src0^2 + Src1     (Src0 may be PSUM, Src1 SBUF)
      SQ_ANT:     out = Src0^2            (PSUM -> SBUF)
    Only one non-scalar PSUM input per instruction (HW limit)."""
    global _OPS
    if _OPS is not None:
        return _OPS
    from concourse import dve_ops as dvo
    from concourse.dve_spec import Spec, Src0, Src1, sq, lower
    from concourse.dve_table_gen import DveOpSpec

    specs = {
        "SQSUM_ANT": Spec(body=sq(Src0) + sq(Src1),
                          reference=lambda in0, in1: in0 * in0 + in1 * in1),
        "SQPLUS_ANT": Spec(body=sq(Src0) + Src1,
                           reference=lambda in0, in1: in0 * in0 + in1),
        "SQ_ANT": Spec(body=sq(Src0),
                       reference=lambda in0: in0 * in0),
    }
    _OPS = {}
    have = {op.name: op for op in dvo.OPS}
    for name, spec in specs.items():
        if name in have:
            _OPS[name] = have[name]
            continue
        shas = {}
        for ver in ("v3", "v4"):
            tmp = DveOpSpec(name=name, opcode=0,
                            uops=lower(spec, ver=ver),
                            rd1_en=name != "SQ_ANT")
            shas[ver] = tmp.sha(ver)
        op = dvo.DveOp(name, spec, subdim=False, uops_sha=shas)
        dvo.OPS.append(op)
        dvo.CUSTOM_DVE_SPECS[op.name] = op.spec
        dvo._SUB_OPCODE_FOR_NAME[op.name] = (
            dvo._CUSTOM_DVE_ROW_BASE + len(dvo.OPS) - 1)
        assert max(dvo._SUB_OPCODE_FOR_NAME.values()) < 0x20
        _OPS[name] = op
    return _OPS


def build_nc(passes=1):
    key = (passes, SKIP_PAIRS, SKIP_EVENTS, USE_DRSW)
    if key in _NC_CACHE:
        return _NC_CACHE[key]
    import concourse.bacc as bacc
    import concourse.mybir as mybir
    import concourse.tile as tile

    f32 = mybir.dt.float32
    bf16 = mybir.dt.bfloat16
    fp8 = mybir.dt.float8e4
    Alu = mybir.AluOpType
    Act = mybir.ActivationFunctionType
    DR = mybir.MatmulPerfMode.DoubleRow
    ops = _register_ops()
    sqsum = ops["SQSUM_ANT"]
    sqplus = ops["SQPLUS_ANT"]
    sqo = ops["SQ_ANT"]

    nc = bacc.Bacc(trn_type="TRN2")

    ev_dram = nc.dram_tensor("ev_all", [EV_TILES, 128, NEV, EV_N], fp8,
                             kind="ExternalInput")
    pr_dram = nc.dram_tensor("pr_all", [128, NPR, PR_N], fp8,
                             kind="ExternalInput")
    WCOL = 256 if USE_DRSW else 128
    diag_dram = nc.dram_tensor("tp_diag", [128, EV_TILES, WCOL], fp8,
                               kind="ExternalInput")
    ident_dram = nc.dram_tensor("ident2", [128, WCOL], fp8,
                                kind="ExternalInput")
    taus_dram = nc.dram_tensor("taus", [128, R + 3], f32, kind="ExternalInput")
    beta_dram = nc.dram_tensor("betab", [128, 1], f32, kind="ExternalInput")
    out_dram = nc.dram_tensor("partials", [128, 12], f32, kind="ExternalOutput")

    with tile.TileContext(nc) as tc:
        with (
            tc.tile_pool(name="evin", bufs=3) as evin,
            tc.tile_pool(name="work", bufs=4) as work,
            tc.tile_pool(name="prp", bufs=1) as prp,
            tc.tile_pool(name="accp", bufs=1) as accp,
            tc.tile_pool(name="psum", bufs=4, space="PSUM") as psum,
        ):
            acc = accp.tile([128, 12], f32)
            taus = accp.tile([128, R + 3], f32)
            betab = accp.tile([128, 1], f32)
            ident2 = accp.tile([128, WCOL], fp8)
            diags = accp.tile([128, EV_TILES, WCOL], fp8)

            # loop-invariant preamble: const DMAs hoisted out of the
            # passes loop, plus a dummy Sqrt that pins the ACT table set to
            # sqrt_and_others on every path into the loop body (Square is in
            # that set too), so the body needs no per-pass reload
            nc.scalar.dma_start(ident2[:], ident_dram.ap()[:])
            nc.scalar.dma_start(diags[:], diag_dram.ap()[:])
            nc.scalar.dma_start(taus[:], taus_dram.ap()[:])
            nc.scalar.dma_start(betab[:], beta_dram.ap()[:])
            nc.scalar.activation(
                accp.tile([128, 1], f32, name="sqdum")[:],
                taus[:, 0:1], Act.Sqrt)

            def body():
                nc.vector.memset(acc[:], 0.0)
                prt = prp.tile([128, NPR, PR_N], fp8, name="prt")

                def ev_dma(t):
                    evt = evin.tile([128, NEV, EV_N], fp8, tag="evt", name="evt")
                    nc.sync.dma_start(evt[:], ev_dram.ap()[t])
                    return evt

                qpair = [None]

                def ev_compute(t, evt):
                    if t % 2 == 0:
                        qpair[0] = work.tile([128, 2, EV_N], bf16, tag="q",
                                             name="q")
                    q = qpair[0]
                    D = diags[:, t]
                    for ci, c0 in enumerate((0, HALF0)):
                        w = HALF0 if c0 == 0 else EV_N - HALF0
                        sl = slice(c0, c0 + w)
                        h = 2 * t + ci
                        axp = psum.tile([128, w], f32, tag="axp", name="axp")
                        ayp = psum.tile([128, w], f32, tag="ayp", name="ayp")
                        xs = work.tile([128, 1, w], bf16, tag="xs", name="xs")
                        if USE_DRSW:
                            DRSW = mybir.MatmulPerfMode.DoubleRowSwInterleave
                            nc.tensor.matmul(axp[:], ident2[:],
                                             evt[:, 0:2, sl], start=True,
                                             stop=False, perf_mode=DRSW)
                            nc.tensor.matmul(axp[:], D, evt[:, 4:6, sl],
                                             start=False, stop=True,
                                             perf_mode=DRSW)
                            nc.tensor.matmul(ayp[:], ident2[:],
                                             evt[:, 2:4, sl], start=True,
                                             stop=False, perf_mode=DRSW)
                            nc.tensor.matmul(ayp[:], D, evt[:, 6:8, sl],
                                             start=False, stop=True,
                                             perf_mode=DRSW)
                        else:
                            nc.tensor.matmul(axp[:], ident2[:], evt[:, 0, sl],
                                             start=True, stop=False)
                            nc.tensor.matmul(axp[:], ident2[:], evt[:, 1, sl],
                                             start=False, stop=False)
                            nc.tensor.matmul(axp[:], D, evt[:, 4, sl],
                                             start=False, stop=False)
                            nc.tensor.matmul(axp[:], D, evt[:, 5, sl],
                                             start=False, stop=True)
                            nc.tensor.matmul(ayp[:], ident2[:], evt[:, 2, sl],
                                             start=True, stop=False)
                            nc.tensor.matmul(ayp[:], ident2[:], evt[:, 3, sl],
                                             start=False, stop=False)
                            nc.tensor.matmul(ayp[:], D, evt[:, 6, sl],
                                             start=False, stop=False)
                            nc.tensor.matmul(ayp[:], D, evt[:, 7, sl],
                                             start=False, stop=True)
                        if h % 2 == 0:
                            nc.scalar.activation(xs[:, 0], axp[:], Act.Square)
                        else:
                            nc.vector._custom_dve(sqo, out=xs[:, 0],
                                                  in0=axp[:])
                        nc.vector._custom_dve(sqplus, out=q[:, t % 2, sl],
                                              in0=ayp[:], in1=xs[:])
                    if t % 2 == 1:
                        nc.scalar.activation(
                            work.tile([128, 2, EV_N], bf16, tag="d",
                                      name="d")[:],
                            q[:], Act.Sqrt, accum_out=acc[:, t // 2:t // 2 + 1])

                evt0 = ev_dma(0)
                nc.sync.dma_start(prt[:], pr_dram.ap()[:])

                if SKIP_PAIRS:
                    if not SKIP_EVENTS:
                        ev_compute(0, evt0)
                        for _t in range(1, EV_TILES):
                            ev_compute(_t, ev_dma(_t))
                    nc.sync.dma_start(out_dram.ap()[:], acc[:])
                    return

                def ps(n):
                    return prt[:, EV_S[n], :]

                pdzx = prp.tile([128, PR_N], bf16, name="pdzx")
                pdzy = prp.tile([128, 1, PR_N], bf16, name="pdzy")
                pdvx = prp.tile([128, PR_N], bf16, name="pdvx")
                pdvy = prp.tile([128, 1, PR_N], bf16, name="pdvy")
                t1 = prp.tile([128, PR_N], bf16, name="t1")
                t2 = prp.tile([128, PR_N], bf16, name="t2")
                A = prp.tile([128, PR_N], bf16, name="A")
                B2 = prp.tile([128, PR_N], bf16, name="B2")
                C = prp.tile([128, PR_N], bf16, name="C")
                qrs = prp.tile([128, R, PR_N], bf16, name="qrs")
                s1s = prp.tile([128, R, PR_N], bf16, name="s1s")
                drs = prp.tile([128, R, PR_N], bf16, name="drs")
                ers = prp.tile([128, R, PR_N], bf16, name="ers")

                def EV(t):
                    if not SKIP_EVENTS:
                        ev_compute(t, evt0 if t == 0 else ev_dma(t))

                EV(0)
                # pair dz/dv adds on Pool (only engine work they block is pairs)
                nc.gpsimd.tensor_tensor(pdzx[:], ps("zux"), ps("zvnx"), Alu.add)
                nc.gpsimd.tensor_tensor(pdzy[:, 0], ps("zuy"), ps("zvny"), Alu.add)
                nc.gpsimd.tensor_tensor(pdvx[:], ps("vux"), ps("vvnx"), Alu.add)
                nc.gpsimd.tensor_tensor(pdvy[:, 0], ps("vuy"), ps("vvny"), Alu.add)
                # A/C via fused sqsum + B2 on DVE, slotted into tile-0 slack
                nc.vector._custom_dve(sqsum, out=A[:], in0=pdzx[:], in1=pdzy[:])
                nc.vector._custom_dve(sqsum, out=C[:], in0=pdvx[:], in1=pdvy[:])
                nc.gpsimd.tensor_tensor(t1[:], pdzx[:], pdvx[:], Alu.mult)
                nc.gpsimd.tensor_tensor(t2[:], pdzy[:, 0], pdvy[:, 0], Alu.mult)
                nc.gpsimd.tensor_tensor(B2[:], t1[:], t2[:], Alu.add)
                nc.vector.tensor_scalar(B2[:], B2[:], 2.0, None, Alu.mult)
                # Riemann q_r: quadratic in tau_r on a uniform grid, so
                # q_{r+1} = q_r + Delta_r, Delta_{r+1} = Delta_r + 2Ch^2
                # (two bf16 TT adds per r instead of 1x-mode STTs)
                dlt = prp.tile([128, PR_N], bf16, name="dlt")
                G = prp.tile([128, PR_N], bf16, name="G")
                EV(1)
                # q_0 = (C*tau0 + B2)*tau0 + A   (Horner)
                nc.vector.scalar_tensor_tensor(
                    s1s[:, 0, :], C[:], taus[:, 0:1], B2[:], Alu.mult, Alu.add)
                nc.vector.scalar_tensor_tensor(
                    qrs[:, 0, :], s1s[:, 0, :], taus[:, 0:1], A[:],
                    Alu.mult, Alu.add)
                # uniform grid: Delta_0 = h*(B2 + (2*tau0+h)*C), G = 2h^2*C
                # taus cols: [R] = h, [R+1] = 2*tau0+h, [R+2] = 2h
                nc.vector.scalar_tensor_tensor(
                    s1s[:, 1, :], C[:], taus[:, R + 1:R + 2], B2[:],
                    Alu.mult, Alu.add)
                nc.vector.tensor_scalar(dlt[:], s1s[:, 1, :],
                                        taus[:, R:R + 1], None, Alu.mult)
                nc.vector.tensor_scalar(G[:], C[:], taus[:, R:R + 1],
                                        taus[:, R + 2:R + 3], Alu.mult,
                                        Alu.mult)
                EV(2)
                for r in range(1, 5):
                    nc.vector.tensor_tensor(qrs[:, r, :], qrs[:, r - 1, :],
                                            dlt[:], Alu.add)
                    nc.vector.tensor_tensor(dlt[:], dlt[:], G[:], Alu.add)
                EV(3)
                for r in range(5, R):
                    nc.vector.tensor_tensor(qrs[:, r, :], qrs[:, r - 1, :],
                                            dlt[:], Alu.add)
                    if r < R - 1:
                        nc.vector.tensor_tensor(dlt[:], dlt[:], G[:], Alu.add)
                nc.vector.tensor_scalar_max(qrs[:], qrs[:], 0.0)
                nc.scalar.activation(drs[:], qrs[:], Act.Sqrt)
                EV(4)
                nc.scalar.activation(ers[:], drs[:], Act.Exp,
                                     bias=betab[:, 0:1], scale=-1.0 / SCALE,
                                     accum_out=acc[:, 8:9])
                for _t in range(5, EV_TILES):
                    EV(_t)
                nc.sync.dma_start(out_dram.ap()[:], acc[:])

            if passes == 1:
                body()
            else:
                with tc.For_i(0, passes):
                    body()
    nc.finalize()
    _NC_CACHE[key] = nc
    return nc


def _host_prepare(beta, z0, v0, u, v, event_times, nu, nv, t0, tn):
    """Shard + sort + gather inputs into per-core DMA-ready arrays.

    Host work: sort/index gather, per-table scale/negate/cast, and
    per-row median-t selection. No per-event arithmetic."""
    z0 = np.asarray(z0, dtype=np.float32)
    v0 = np.asarray(v0, dtype=np.float32)
    zs = np.clip(z0 * SCALE, -PAD_Z, PAD_Z)
    vs = np.clip(v0 * SCALE, -PAD_Z, PAD_Z)
    zu8 = zs.astype(FP8).view(np.uint8)       # u-side:  +64*z0
    vu8 = vs.astype(FP8).view(np.uint8)
    zv8 = (-zs).astype(FP8).view(np.uint8)    # v-side:  -64*z0
    vv8 = (-vs).astype(FP8).view(np.uint8)

    u = np.asarray(u).astype(np.int64, copy=False)
    v = np.asarray(v).astype(np.int64, copy=False)
    nu = np.asarray(nu).astype(np.int64, copy=False)
    nv = np.asarray(nv).astype(np.int64, copy=False)
    tarr = np.asarray(event_times, dtype=np.float32)

    order = np.argsort(tarr, kind="stable")
    u = u[order]; v = v[order]; ts_sorted = tarr[order]

    t0f = float(np.asarray(t0)); tnf = float(np.asarray(tn))
    dt = (tnf - t0f) / R
    taus = (t0f + (np.arange(R, dtype=np.float64) + 0.5) * dt).astype(np.float32)
    tx = np.concatenate([taus, np.array(
        [dt, 2 * taus[0] + dt, 2 * dt], dtype=np.float32)])
    taus_arr = np.broadcast_to(tx[None, :], (128, R + 3)).copy()
    betaf = float(np.asarray(beta).reshape(-1)[0])
    beta_arr = np.full((128, 1), betaf, dtype=np.float32)
    one8 = np.array(1.0, dtype=np.float32).astype(FP8).view(np.uint8).item()
    if USE_DRSW:
        # SwInterleave phys layout: W_logical_i[k, m] = phys[k, 2*(127-m)+i]
        ident2 = np.zeros((128, 256), dtype=np.uint8)
        mm = np.arange(128)
        ident2[mm, 2 * (127 - mm)] = one8
        ident2[mm, 2 * (127 - mm) + 1] = one8
    else:
        ident2 = np.eye(128, dtype=np.float32).astype(FP8).view(np.uint8)
    pad8 = np.array(PAD_Z, dtype=np.float32).astype(FP8).view(np.uint8).item()

    in_maps = []
    for c in range(N_CORES):
        es = slice(c * E_CORE, (c + 1) * E_CORE)
        psl = slice(c * P_CORE, (c + 1) * P_CORE)
        uc, vc = u[es], v[es]
        nuc, nvc = nu[psl], nv[psl]

        ev = np.zeros((NEV, E_PAD), dtype=np.uint8)
        ev[EV_S["zux"], :E_CORE] = zu8[uc, 0]
        ev[EV_S["zuy"], :E_CORE] = zu8[uc, 1]
        ev[EV_S["zvnx"], :E_CORE] = zv8[vc, 0]
        ev[EV_S["zvny"], :E_CORE] = zv8[vc, 1]
        ev[EV_S["vux"], :E_CORE] = vu8[uc, 0]
        ev[EV_S["vuy"], :E_CORE] = vu8[uc, 1]
        ev[EV_S["vvnx"], :E_CORE] = vv8[vc, 0]
        ev[EV_S["vvny"], :E_CORE] = vv8[vc, 1]
        # [NEV, E_PAD] -> [EV_TILES, 128, NEV, EV_N]
        ev = ev.reshape(NEV, EV_TILES, 128, EV_N).transpose(1, 2, 0, 3).copy()

        tc_core = np.zeros(E_PAD, dtype=np.float32)
        tc_core[:E_CORE] = ts_sorted[es]
        # same event->row map as ev: row-median t per (tile, partition)
        tp = tc_core.reshape(EV_TILES, 128, EV_N)[:, :, EV_N // 2]  # [T,128]
        tp8 = tp.astype(FP8).view(np.uint8)
        idx = np.arange(128)
        if USE_DRSW:
            diag = np.zeros((128, EV_TILES, 256), dtype=np.uint8)
            for T in range(EV_TILES):
                diag[idx, T, 2 * (127 - idx)] = tp8[T]
                diag[idx, T, 2 * (127 - idx) + 1] = tp8[T]
        else:
            diag = np.zeros((128, EV_TILES, 128), dtype=np.uint8)
            for T in range(EV_TILES):
                diag[idx, T, idx] = tp8[T]

        pr = np.zeros((NPR, PR_N * 128), dtype=np.uint8)
        pr[EV_S["zux"], P_CORE:] = pad8
        pr[EV_S["zvnx"], P_CORE:] = pad8
        pr[EV_S["zux"], :P_CORE] = zu8[nuc, 0]
        pr[EV_S["zuy"], :P_CORE] = zu8[nuc, 1]
        pr[EV_S["zvnx"], :P_CORE] = zv8[nvc, 0]
        pr[EV_S["zvny"], :P_CORE] = zv8[nvc, 1]
        pr[EV_S["vux"], :P_CORE] = vu8[nuc, 0]
        pr[EV_S["vuy"], :P_CORE] = vu8[nuc, 1]
        pr[EV_S["vvnx"], :P_CORE] = vv8[nvc, 0]
        pr[EV_S["vvny"], :P_CORE] = vv8[nvc, 1]
        pr = pr.reshape(NPR, 128, PR_N).transpose(1, 0, 2).copy()

        m = {"ev_all": ev.view(FP8), "pr_all": pr.view(FP8),
             "tp_diag": diag.view(FP8), "ident2": ident2.view(FP8),
             "taus": taus_arr, "betab": beta_arr}
        in_maps.append(m)
    return in_maps, betaf, dt


def _combine(results, betaf, dt):
    d_sum = 0.0
    e_sum = 0.0
    for res in results:
        p = res["partials"].astype(np.float64)
        d_sum += p[:, 0:EV_TILES // 2].sum()
        e_sum += p[:, 8].sum()
    # exact removal of the pad-pair contribution (d_scaled = 2*PAD_Z)
    n_pad_pairs = N_CORES * (PR_N * 128 - P_CORE)
    e_sum -= n_pad_pairs * R * np.exp(betaf - 2.0 * PAD_Z / SCALE)
    val = N_EVENTS * betaf - d_sum / SCALE - e_sum * dt
    return np.array([[val]], dtype=np.float32)


def kernel(beta, z0, v0, u, v, event_times, nu, nv, t0, tn):
    from concourse import bass_utils
    in_maps, betaf, dt = _host_prepare(beta, z0, v0, u, v, event_times,
                                       nu, nv, t0, tn)
    nc = build_nc(passes=1)
    res = bass_utils.run_bass_kernel_spmd(nc, in_maps,
                                          core_ids=list(range(N_CORES)))
    return _combine(res.results, betaf, dt)

